# revision 20
# baseline (speedup 1.0000x reference)
"""Trainium2 Bass kernel for nn_LossCR (segment-reduce + dual CE loss).

Strategy (data-parallel over N x H/2 -> 8 shards of 131072 pixels):
  Host packs, per core, a pixel-major fp8(e4m3) "combo" tensor
  (128 lanes, 1024 chunks, 150 cols) = [z(128) | label_int8(1) |
  preds(21)].  Labels ride inside the combo stream (read on device via an
  int8 bitcast view of col 128) so every DMA queue gets identical work --
  a separate labels DMA skewed one queue by ~6us.  The DRAM row is padded
  by 1024B so the row stride stays 1024 mod 4096 (stride 0 mod 4096 made
  some queues' descriptors ~2x slower from HBM channel conflicts).
  Device, per 256-pixel double-chunk: one fp8 DoubleRow matmul
      S(21,150) += onehot(128,2,21)^T @ combo(128,2,150)
  PSUM-accumulated over all 512 double-chunks (the label col feeds an
  ignored S column).  Onehot built on DVE (is_equal vs int8 iota, fp8
  out, 32-col slabs whose junk pads feed only discarded S rows);
  exp(preds) on ACT; per-pixel sumexp reduce on DVE (bf16), emitted one
  piece behind the is_eq so the DVE never idles waiting for ACT;
  ln(sumexp) on ACT with accum_out giving the per-partition row sums for
  free.  A single manually-loaded ACT table holds BOTH exp and ln (no
  mid-kernel table switch).  Counts come from a host-side bincount.
  Piece schedule: 16/16/32/64-chunk lead-in (early DVE start), six
  128-chunk body pieces (deep DMA ring), 32/32/32/16/16-chunk tail
  (short final DMA->exp->reduce->ln->out chain).
  Host: sum 8 partial (21,152) blocks ([S | slse_a slse_b] row 0),
  tiny (21,) softmax math in f64.
"""
import sys

sys.path.insert(0, "/opt/trn_rl_repo")
import numpy as np
import ml_dtypes
import concourse.bacc as bacc
import concourse.mybir as mybir
import concourse.tile as tile
from concourse import bass_utils
from concourse._compat import axon_active
from concourse.hw_specs import get_activation_tables

f32 = mybir.dt.float32
bf16 = mybir.dt.bfloat16
f8 = mybir.dt.float8e4
AF = mybir.ActivationFunctionType
ALU = mybir.AluOpType
AX = mybir.AxisListType
PM = mybir.MatmulPerfMode

N, C, H, W, D = 4, 21, 512, 512, 128
NCORES = 8
PIX = N * H * W // NCORES      # 131072 pixels per core
CHUNKS = PIX // 128            # 1024
DCHUNKS = CHUNKS // 2          # 512 double-chunks (DoubleRow: 256 px each)
LB = D                         # label byte (int8) at col 128
P0 = D + 1                     # preds start col
COLS = D + 1 + C               # 150
MC = 32                        # onehot cols per slab (DoubleRow needs 32-aligned)
LS = 0.1                       # label smoothing
LAMBDA_REG = 0.4
PIECES = ([(0, 16), (16, 16), (32, 32), (64, 64)]
          + [(128 + g * 128, 128) for g in range(6)]
          + [(896, 32), (928, 32), (960, 32), (992, 16), (1008, 16)])
TAILC = 896                    # chunks covered before the tail pieces

_nc_cache = None


def _build():
    global _nc_cache
    if _nc_cache is not None:
        return _nc_cache
    nc = bacc.Bacc("TRN2", target_bir_lowering=False, debug=not axon_active())
    # 1024B row pad keeps the DRAM row stride at 1024 mod 4096 so
    # partition rows spread across HBM channels (stride 0 mod 4096 made
    # some DMA queues' descriptors run ~2x slower from channel conflicts)
    cbd = nc.dram_tensor("combo", [128, CHUNKS * COLS + 1024], f8,
                         kind="ExternalInput").ap()
    outd = nc.dram_tensor("out", [C, COLS + 2], f32, kind="ExternalOutput").ap()

    with tile.TileContext(nc) as tc:
        with tc.tile_pool(name="const", bufs=1) as cpool, \
             tc.tile_pool(name="work", bufs=3) as wpool, \
             tc.tile_pool(name="cb", bufs=4) as cbpool, \
             tc.tile_pool(name="cbs", bufs=3) as cbspool, \
             tc.tile_pool(name="ps", bufs=1, space="PSUM") as pspool, \
             tc.tile_pool(name="acc", bufs=1, space="PSUM") as apool:
            # Preload the one ACT table that holds BOTH exp and ln so the
            # auto-inserter never needs a mid-kernel table switch.
            tabs = list(get_activation_tables(nc.m.arch).values())
            tid = next(i for i, s in enumerate(tabs)
                       if AF.Exp in s and AF.Ln in s)
            nc.scalar.add_instruction(mybir.InstLoadActFuncSet(
                name=nc.get_next_instruction_name(), ins=[], outs=[],
                act_func_set_id=tid))

            iota_sb = cpool.tile([128, MC], mybir.dt.int8, tag="iota_sb")
            nc.gpsimd.iota(iota_sb[:], [[1, MC]], channel_multiplier=0,
                           allow_small_or_imprecise_dtypes=True)
            ones_sb = cpool.tile([128, 1], f32, tag="ones_sb")
            nc.vector.memset(ones_sb[:], 1.0)
            sumexp_buf = cpool.tile([128, CHUNKS], bf16, tag="sumexp_buf")
            lse = cpool.tile([128, CHUNKS], f32, tag="lse")
            red = cpool.tile([128, 2], f32, tag="red")
            S_ps = apool.tile([MC, COLS], f32, tag="S_ps")

            pending_add = None

            def _emit_add(pa):
                ac0, an, aex = pa
                with nc.allow_low_precision("bf16 sumexp; rel tol 2e-2"):
                    nc.vector.tensor_reduce(
                        sumexp_buf[:, ac0:ac0 + an],
                        aex[:].rearrange("p (c k) -> p c k", k=C),
                        axis=AX.X, op=ALU.add)

            for c0, n in PIECES:
                pool = cbpool if n == 128 else cbspool
                cb = pool.tile([128, n * COLS], f8, tag=f"cb{n}")
                nc.sync.dma_start(cb[:], cbd[:, c0 * COLS:(c0 + n) * COLS])
                cb_r = cb[:].rearrange("p (c m) -> p c m", m=COLS)
                # labels as an int8 view of col 128 of each chunk
                labv = cb[:].bitcast(mybir.dt.int8).rearrange(
                    "p (c k) -> p c k", k=COLS)[:, :, LB:LB + 1]
                # onehot(label) for the piece's n*128 pixels, fp8 out
                # pad cols 21:32 are left as garbage: each stationary column
                # only feeds its own S_ps row, and rows 21:31 are discarded
                oh = wpool.tile([128, n * MC], f8, tag=f"oh{n}")
                oh_r = oh[:].rearrange("p (c k) -> p c k", k=MC)
                nc.vector.tensor_tensor(
                    oh_r[:, :, 0:C],
                    iota_sb[:, 0:C].unsqueeze(1).broadcast_to([128, n, C]),
                    labv.broadcast_to([128, n, C]),
                    op=ALU.is_equal)
                # CE pieces: exp(preds) on ACT; the piece's sumexp
                # reduce is deferred one iteration so the DVE never idles
                # waiting for ACT (eq(p) runs while exp(p) is in flight)
                ex = wpool.tile([128, n * C], bf16, tag=f"ex{n}")
                nc.scalar.activation(
                    ex[:].rearrange("p (c k) -> p c k", k=C),
                    cb_r[:, :, P0:P0 + C], AF.Exp)
                if pending_add is not None:
                    _emit_add(pending_add)
                pending_add = (c0, n, ex)
                # segment sums: S += oh^T @ [z | lab | preds], 2 chunks/matmul
                # (label-byte cols land in S cols 128:130, ignored on host)
                for i in range(n // 2):
                    dc = c0 // 2 + i
                    nc.tensor.matmul(
                        S_ps[:],
                        oh_r[:, 2 * i:2 * i + 2, :],
                        cb_r[:, 2 * i:2 * i + 2, 0:COLS],
                        start=(dc == 0), stop=(dc == DCHUNKS - 1),
                        perf_mode=PM.DoubleRow)

            if pending_add is not None:
                _emit_add(pending_add)
            # --- epilogue: slse = sum(ln(sumexp)); accum_out gives the
            # per-partition row sums for free (split A/B so the tail piece
            # dependency chain stays short)
            nc.scalar.activation(lse[:, 0:TAILC], sumexp_buf[:, 0:TAILC],
                                 AF.Ln, accum_out=red[:, 0:1])
            nc.scalar.activation(lse[:, TAILC:CHUNKS],
                                 sumexp_buf[:, TAILC:CHUNKS],
                                 AF.Ln, accum_out=red[:, 1:2])
            fin_ps = pspool.tile([1, 2], f32, tag="fin_ps")
            nc.tensor.matmul(fin_ps[:], ones_sb[:], red[:], start=True,
                             stop=True)
            out_sb = cpool.tile([C, COLS + 2], f32, tag="out_sb")
            nc.scalar.copy(out_sb[:, 0:COLS], S_ps[0:C, :])
            nc.scalar.copy(out_sb[0:1, COLS:COLS + 2], fin_ps[:])
            nc.sync.dma_start(outd, out_sb[:])

    nc.compile()
    _nc_cache = nc
    return nc


_F8 = ml_dtypes.float8_e4m3
_BF16 = ml_dtypes.bfloat16


def _make_in_maps(preds, labels, z, W_star):
    in_maps = []
    for i in range(NCORES):
        n, h0 = i // 2, (i % 2) * (H // 2)
        # pixel p = h*512 + w -> chunk = h*4 + w//128, lane = w%128
        zc = z[n, :, h0:h0 + H // 2, :].reshape(D, 256, 4, 128)
        zc = zc.transpose(3, 1, 2, 0).reshape(128, CHUNKS, D)
        pc = preds[n, :, h0:h0 + H // 2, :].reshape(C, 256, 4, 128)
        pc = pc.transpose(3, 1, 2, 0).reshape(128, CHUNKS, C)
        lc = labels[n, h0:h0 + H // 2, :].reshape(256, 4, 128)
        lc = lc.transpose(2, 0, 1).reshape(128, CHUNKS)
        combo = np.empty((128, CHUNKS, COLS), dtype=_F8)
        combo[:, :, 0:D] = zc.astype(_F8)
        combo[:, :, LB] = lc.astype(np.int8).view(np.uint8).view(_F8)
        combo[:, :, P0:P0 + C] = pc.astype(_F8)
        cpad = np.zeros((128, CHUNKS * COLS + 1024), dtype=_F8)
        cpad[:, :CHUNKS * COLS] = combo.reshape(128, CHUNKS * COLS)
        in_maps.append(dict(combo=cpad))
    return in_maps


def _combine(outs, W_star, counts, npix):
    """outs: 8x(21,153) blocks ([S_z|lab junk|SP|slse_a slse_b]) -> loss."""
    tot = np.sum([o.astype(np.float64) for o in outs], axis=0)
    S_z = tot[0:C, 0:D]
    SP = tot[0:C, P0:P0 + C]
    slse = tot[0, COLS] + tot[0, COLS + 1]
    ssx = SP.sum()
    sem = (slse - (1.0 - LS) * np.trace(SP) - (LS / C) * ssx) / npix
    cnt = counts.astype(np.float64)
    Zbar = np.where(cnt[:, None] > 0, S_z / np.maximum(cnt, 1.0)[:, None], 0.0)
    logits = Zbar @ W_star.astype(np.float64)
    m = logits.max(axis=1, keepdims=True)
    lse_r = m[:, 0] + np.log(np.exp(logits - m).sum(axis=1))
    lcr = np.mean(lse_r - (1.0 - LS) * np.diag(logits)
                  - (LS / C) * logits.sum(axis=1))
    return np.float32(LAMBDA_REG * lcr + sem)


def kernel(preds, labels, labels_depth, z, W_star):
    nc = _build()
    in_maps = _make_in_maps(preds, labels, z, W_star)
    res = bass_utils.run_bass_kernel_spmd(nc, in_maps,
                                          core_ids=list(range(NCORES)))
    counts = np.bincount(labels.reshape(-1), minlength=C).astype(np.float64)
    return _combine([r["out"] for r in res.results], W_star, counts,
                    float(labels.size))


if __name__ == "__main__":
    rng = np.random.default_rng(0)
    preds = rng.standard_normal((N, C, H, W), dtype=np.float32)
    labels = rng.integers(0, C, size=(N, H, W)).astype(np.int32)
    ld = rng.standard_normal((N, H, W), dtype=np.float32)
    z = rng.standard_normal((N, D, H, W), dtype=np.float32)
    Wst = rng.standard_normal((D, C), dtype=np.float32) * 0.3
    print("loss:", kernel(preds, labels, ld, z, Wst))


# revision 21
# speedup vs baseline: 1.1137x; 1.1137x over previous
"""Trainium2 Bass kernel for nn_LossCR (segment-reduce + dual CE loss).

Strategy (data-parallel over N x H/2 -> 8 shards of 131072 pixels):
  Host packs, per core, a pixel-major fp8(e4m3) "combo" tensor
  (128 lanes, 1024 chunks, 150 cols) = [z(128) | label_int8(1) |
  preds(21)].  Labels ride inside the combo stream (read on device via an
  int8 bitcast view of col 128) so every DMA queue gets identical work --
  a separate labels DMA skewed one queue by ~6us.  The DRAM row is padded
  by 1024B so the row stride stays 1024 mod 4096 (stride 0 mod 4096 made
  some queues' descriptors ~2x slower from HBM channel conflicts).
  Device, per 256-pixel double-chunk: one fp8 DoubleRow matmul
      S(21,150) += onehot(128,2,21)^T @ combo(128,2,150)
  PSUM-accumulated over all 512 double-chunks (the label col feeds an
  ignored S column).  Onehot built on DVE (is_equal vs int8 iota, fp8
  out, 32-col slabs whose junk pads feed only discarded S rows);
  exp(preds) on ACT; per-pixel sumexp reduce on DVE (bf16), emitted one
  piece behind the is_eq so the DVE never idles waiting for ACT;
  ln(sumexp) on ACT with accum_out giving the per-partition row sums for
  free.  A single manually-loaded ACT table holds BOTH exp and ln (no
  mid-kernel table switch).  Counts come from a host-side bincount.
  Piece schedule: 16/16-chunk lead-in (early DVE start), fifteen
  64-chunk body pieces on a deep 8-buffer DMA ring (absorbs HBM
  jitter), 16/16-chunk tail (short final DMA->exp->reduce->ln->out
  chain).
  Host: sum 8 partial (21,152) blocks ([S | slse_a slse_b] row 0),
  tiny (21,) softmax math in f64.
"""
import sys

sys.path.insert(0, "/opt/trn_rl_repo")
import numpy as np
import ml_dtypes
import concourse.bacc as bacc
import concourse.mybir as mybir
import concourse.tile as tile
from concourse import bass_utils
from concourse._compat import axon_active
from concourse.hw_specs import get_activation_tables

f32 = mybir.dt.float32
bf16 = mybir.dt.bfloat16
f8 = mybir.dt.float8e4
AF = mybir.ActivationFunctionType
ALU = mybir.AluOpType
AX = mybir.AxisListType
PM = mybir.MatmulPerfMode

N, C, H, W, D = 4, 21, 512, 512, 128
NCORES = 8
PIX = N * H * W // NCORES      # 131072 pixels per core
CHUNKS = PIX // 128            # 1024
DCHUNKS = CHUNKS // 2          # 512 double-chunks (DoubleRow: 256 px each)
LB = D                         # label byte (int8) at col 128
P0 = D + 1                     # preds start col
COLS = D + 1 + C               # 150
MC = 32                        # onehot cols per slab (DoubleRow needs 32-aligned)
LS = 0.1                       # label smoothing
LAMBDA_REG = 0.4
PIECES = ([(0, 16), (16, 16)]
          + [(32 + g * 64, 64) for g in range(15)]
          + [(992, 16), (1008, 16)])
TAILC = 992                    # chunks covered before the tail pieces

_nc_cache = None


def _build():
    global _nc_cache
    if _nc_cache is not None:
        return _nc_cache
    nc = bacc.Bacc("TRN2", target_bir_lowering=False, debug=not axon_active())
    # 1024B row pad keeps the DRAM row stride at 1024 mod 4096 so
    # partition rows spread across HBM channels (stride 0 mod 4096 made
    # some DMA queues' descriptors run ~2x slower from channel conflicts)
    cbd = nc.dram_tensor("combo", [128, CHUNKS * COLS + 1024], f8,
                         kind="ExternalInput").ap()
    outd = nc.dram_tensor("out", [C, COLS + 2], f32, kind="ExternalOutput").ap()

    with tile.TileContext(nc) as tc:
        with tc.tile_pool(name="const", bufs=1) as cpool, \
             tc.tile_pool(name="work", bufs=4) as wpool, \
             tc.tile_pool(name="cb", bufs=8) as cbpool, \
             tc.tile_pool(name="cbs", bufs=3) as cbspool, \
             tc.tile_pool(name="ps", bufs=1, space="PSUM") as pspool, \
             tc.tile_pool(name="acc", bufs=1, space="PSUM") as apool:
            # Preload the one ACT table that holds BOTH exp and ln so the
            # auto-inserter never needs a mid-kernel table switch.
            tabs = list(get_activation_tables(nc.m.arch).values())
            tid = next(i for i, s in enumerate(tabs)
                       if AF.Exp in s and AF.Ln in s)
            nc.scalar.add_instruction(mybir.InstLoadActFuncSet(
                name=nc.get_next_instruction_name(), ins=[], outs=[],
                act_func_set_id=tid))

            iota_sb = cpool.tile([128, MC], mybir.dt.int8, tag="iota_sb")
            nc.gpsimd.iota(iota_sb[:], [[1, MC]], channel_multiplier=0,
                           allow_small_or_imprecise_dtypes=True)
            ones_sb = cpool.tile([128, 1], f32, tag="ones_sb")
            nc.vector.memset(ones_sb[:], 1.0)
            sumexp_buf = cpool.tile([128, CHUNKS], bf16, tag="sumexp_buf")
            lse = cpool.tile([128, CHUNKS], f32, tag="lse")
            red = cpool.tile([128, 2], f32, tag="red")
            S_ps = apool.tile([MC, COLS], f32, tag="S_ps")

            pending_add = None

            def _emit_add(pa):
                ac0, an, aex = pa
                with nc.allow_low_precision("bf16 sumexp; rel tol 2e-2"):
                    nc.vector.tensor_reduce(
                        sumexp_buf[:, ac0:ac0 + an],
                        aex[:].rearrange("p (c k) -> p c k", k=C),
                        axis=AX.X, op=ALU.add)

            for c0, n in PIECES:
                pool = cbpool if n == 64 else cbspool
                cb = pool.tile([128, n * COLS], f8, tag=f"cb{n}")
                nc.sync.dma_start(cb[:], cbd[:, c0 * COLS:(c0 + n) * COLS])
                cb_r = cb[:].rearrange("p (c m) -> p c m", m=COLS)
                # labels as an int8 view of col 128 of each chunk
                labv = cb[:].bitcast(mybir.dt.int8).rearrange(
                    "p (c k) -> p c k", k=COLS)[:, :, LB:LB + 1]
                # onehot(label) for the piece's n*128 pixels, fp8 out
                # pad cols 21:32 are left as garbage: each stationary column
                # only feeds its own S_ps row, and rows 21:31 are discarded
                oh = wpool.tile([128, n * MC], f8, tag=f"oh{n}")
                oh_r = oh[:].rearrange("p (c k) -> p c k", k=MC)
                nc.vector.tensor_tensor(
                    oh_r[:, :, 0:C],
                    iota_sb[:, 0:C].unsqueeze(1).broadcast_to([128, n, C]),
                    labv.broadcast_to([128, n, C]),
                    op=ALU.is_equal)
                # CE pieces: exp(preds) on ACT; the piece's sumexp
                # reduce is deferred one iteration so the DVE never idles
                # waiting for ACT (eq(p) runs while exp(p) is in flight)
                ex = wpool.tile([128, n * C], bf16, tag=f"ex{n}")
                nc.scalar.activation(
                    ex[:].rearrange("p (c k) -> p c k", k=C),
                    cb_r[:, :, P0:P0 + C], AF.Exp)
                if pending_add is not None:
                    _emit_add(pending_add)
                pending_add = (c0, n, ex)
                # segment sums: S += oh^T @ [z | lab | preds], 2 chunks/matmul
                # (label-byte cols land in S cols 128:130, ignored on host)
                for i in range(n // 2):
                    dc = c0 // 2 + i
                    nc.tensor.matmul(
                        S_ps[:],
                        oh_r[:, 2 * i:2 * i + 2, :],
                        cb_r[:, 2 * i:2 * i + 2, 0:COLS],
                        start=(dc == 0), stop=(dc == DCHUNKS - 1),
                        perf_mode=PM.DoubleRow)

            if pending_add is not None:
                _emit_add(pending_add)
            # --- epilogue: slse = sum(ln(sumexp)); accum_out gives the
            # per-partition row sums for free (split A/B so the tail piece
            # dependency chain stays short)
            nc.scalar.activation(lse[:, 0:TAILC], sumexp_buf[:, 0:TAILC],
                                 AF.Ln, accum_out=red[:, 0:1])
            nc.scalar.activation(lse[:, TAILC:CHUNKS],
                                 sumexp_buf[:, TAILC:CHUNKS],
                                 AF.Ln, accum_out=red[:, 1:2])
            fin_ps = pspool.tile([1, 2], f32, tag="fin_ps")
            nc.tensor.matmul(fin_ps[:], ones_sb[:], red[:], start=True,
                             stop=True)
            out_sb = cpool.tile([C, COLS + 2], f32, tag="out_sb")
            nc.scalar.copy(out_sb[:, 0:COLS], S_ps[0:C, :])
            nc.scalar.copy(out_sb[0:1, COLS:COLS + 2], fin_ps[:])
            nc.sync.dma_start(outd, out_sb[:])

    nc.compile()
    _nc_cache = nc
    return nc


_F8 = ml_dtypes.float8_e4m3
_BF16 = ml_dtypes.bfloat16


def _make_in_maps(preds, labels, z, W_star):
    in_maps = []
    for i in range(NCORES):
        n, h0 = i // 2, (i % 2) * (H // 2)
        # pixel p = h*512 + w -> chunk = h*4 + w//128, lane = w%128
        zc = z[n, :, h0:h0 + H // 2, :].reshape(D, 256, 4, 128)
        zc = zc.transpose(3, 1, 2, 0).reshape(128, CHUNKS, D)
        pc = preds[n, :, h0:h0 + H // 2, :].reshape(C, 256, 4, 128)
        pc = pc.transpose(3, 1, 2, 0).reshape(128, CHUNKS, C)
        lc = labels[n, h0:h0 + H // 2, :].reshape(256, 4, 128)
        lc = lc.transpose(2, 0, 1).reshape(128, CHUNKS)
        combo = np.empty((128, CHUNKS, COLS), dtype=_F8)
        combo[:, :, 0:D] = zc.astype(_F8)
        combo[:, :, LB] = lc.astype(np.int8).view(np.uint8).view(_F8)
        combo[:, :, P0:P0 + C] = pc.astype(_F8)
        cpad = np.zeros((128, CHUNKS * COLS + 1024), dtype=_F8)
        cpad[:, :CHUNKS * COLS] = combo.reshape(128, CHUNKS * COLS)
        in_maps.append(dict(combo=cpad))
    return in_maps


def _combine(outs, W_star, counts, npix):
    """outs: 8x(21,153) blocks ([S_z|lab junk|SP|slse_a slse_b]) -> loss."""
    tot = np.sum([o.astype(np.float64) for o in outs], axis=0)
    S_z = tot[0:C, 0:D]
    SP = tot[0:C, P0:P0 + C]
    slse = tot[0, COLS] + tot[0, COLS + 1]
    ssx = SP.sum()
    sem = (slse - (1.0 - LS) * np.trace(SP) - (LS / C) * ssx) / npix
    cnt = counts.astype(np.float64)
    Zbar = np.where(cnt[:, None] > 0, S_z / np.maximum(cnt, 1.0)[:, None], 0.0)
    logits = Zbar @ W_star.astype(np.float64)
    m = logits.max(axis=1, keepdims=True)
    lse_r = m[:, 0] + np.log(np.exp(logits - m).sum(axis=1))
    lcr = np.mean(lse_r - (1.0 - LS) * np.diag(logits)
                  - (LS / C) * logits.sum(axis=1))
    return np.float32(LAMBDA_REG * lcr + sem)


def kernel(preds, labels, labels_depth, z, W_star):
    nc = _build()
    in_maps = _make_in_maps(preds, labels, z, W_star)
    res = bass_utils.run_bass_kernel_spmd(nc, in_maps,
                                          core_ids=list(range(NCORES)))
    counts = np.bincount(labels.reshape(-1), minlength=C).astype(np.float64)
    return _combine([r["out"] for r in res.results], W_star, counts,
                    float(labels.size))


if __name__ == "__main__":
    rng = np.random.default_rng(0)
    preds = rng.standard_normal((N, C, H, W), dtype=np.float32)
    labels = rng.integers(0, C, size=(N, H, W)).astype(np.int32)
    ld = rng.standard_normal((N, H, W), dtype=np.float32)
    z = rng.standard_normal((N, D, H, W), dtype=np.float32)
    Wst = rng.standard_normal((D, C), dtype=np.float32) * 0.3
    print("loss:", kernel(preds, labels, ld, z, Wst))


# revision 22
# speedup vs baseline: 1.1237x; 1.0090x over previous
"""Trainium2 Bass kernel for nn_LossCR (segment-reduce + dual CE loss).

Strategy (data-parallel over N x H/2 -> 8 shards of 131072 pixels):
  Host packs, per core, a pixel-major fp8(e4m3) "combo" tensor
  (128 lanes, 1024 chunks, 150 cols) = [z(128) | label_int8(1) |
  preds(21)].  Labels ride inside the combo stream (read on device via an
  int8 bitcast view of col 128) so every DMA queue gets identical work --
  a separate labels DMA skewed one queue by ~6us.  The DRAM row is padded
  by 1024B so the row stride stays 1024 mod 4096 (stride 0 mod 4096 made
  some queues' descriptors ~2x slower from HBM channel conflicts).
  Device, per 256-pixel double-chunk: one fp8 DoubleRow matmul
      S(21,150) += onehot(128,2,21)^T @ combo(128,2,150)
  PSUM-accumulated over all 512 double-chunks (the label col feeds an
  ignored S column).  Onehot built on DVE (is_equal vs int8 iota, fp8
  out, 32-col slabs whose junk pads feed only discarded S rows);
  exp(preds) on ACT; per-pixel sumexp reduce on DVE (bf16), emitted one
  piece behind the is_eq so the DVE never idles waiting for ACT;
  ln(sumexp) on ACT with accum_out giving the per-partition row sums for
  free.  A single manually-loaded ACT table holds BOTH exp and ln (no
  mid-kernel table switch).  Counts come from a host-side bincount.
  Piece schedule: 16/16-chunk lead-in (early DVE start), fifteen
  64-chunk body pieces on a deep 8-buffer DMA ring (absorbs HBM
  jitter), 16/16-chunk tail (short final DMA->exp->reduce->ln->out
  chain).
  Host: sum 8 partial (21,152) blocks ([S | slse_a slse_b] row 0),
  tiny (21,) softmax math in f64.
"""
import sys

sys.path.insert(0, "/opt/trn_rl_repo")
import numpy as np
import ml_dtypes
import concourse.bacc as bacc
import concourse.mybir as mybir
import concourse.tile as tile
from concourse import bass_utils
from concourse._compat import axon_active
from concourse.hw_specs import get_activation_tables

f32 = mybir.dt.float32
bf16 = mybir.dt.bfloat16
f8 = mybir.dt.float8e4
AF = mybir.ActivationFunctionType
ALU = mybir.AluOpType
AX = mybir.AxisListType
PM = mybir.MatmulPerfMode

N, C, H, W, D = 4, 21, 512, 512, 128
NCORES = 8
PIX = N * H * W // NCORES      # 131072 pixels per core
CHUNKS = PIX // 128            # 1024
DCHUNKS = CHUNKS // 2          # 512 double-chunks (DoubleRow: 256 px each)
LB = D                         # label byte (int8) at col 128
P0 = D + 1                     # preds start col
COLS = D + 1 + C               # 150
MC = 32                        # onehot cols per slab (DoubleRow needs 32-aligned)
LS = 0.1                       # label smoothing
LAMBDA_REG = 0.4
PIECES = ([(0, 16), (16, 16), (32, 32), (64, 32), (96, 32)]
          + [(128 + g * 64, 64) for g in range(13)]
          + [(960, 32), (992, 16), (1008, 16)])
TAILC = 992                    # chunks covered before the tail pieces

_nc_cache = None


def _build():
    global _nc_cache
    if _nc_cache is not None:
        return _nc_cache
    nc = bacc.Bacc("TRN2", target_bir_lowering=False, debug=not axon_active())
    # 1024B row pad keeps the DRAM row stride at 1024 mod 4096 so
    # partition rows spread across HBM channels (stride 0 mod 4096 made
    # some DMA queues' descriptors run ~2x slower from channel conflicts)
    cbd = nc.dram_tensor("combo", [128, CHUNKS * COLS + 1024], f8,
                         kind="ExternalInput").ap()
    outd = nc.dram_tensor("out", [C, COLS + 2], f32, kind="ExternalOutput").ap()

    with tile.TileContext(nc) as tc:
        with tc.tile_pool(name="const", bufs=1) as cpool, \
             tc.tile_pool(name="work", bufs=4) as wpool, \
             tc.tile_pool(name="cb", bufs=8) as cbpool, \
             tc.tile_pool(name="cbs", bufs=3) as cbspool, \
             tc.tile_pool(name="ps", bufs=1, space="PSUM") as pspool, \
             tc.tile_pool(name="acc", bufs=1, space="PSUM") as apool:
            # Preload the one ACT table that holds BOTH exp and ln so the
            # auto-inserter never needs a mid-kernel table switch.
            tabs = list(get_activation_tables(nc.m.arch).values())
            tid = next(i for i, s in enumerate(tabs)
                       if AF.Exp in s and AF.Ln in s)
            nc.scalar.add_instruction(mybir.InstLoadActFuncSet(
                name=nc.get_next_instruction_name(), ins=[], outs=[],
                act_func_set_id=tid))

            iota_sb = cpool.tile([128, MC], mybir.dt.int8, tag="iota_sb")
            nc.gpsimd.iota(iota_sb[:], [[1, MC]], channel_multiplier=0,
                           allow_small_or_imprecise_dtypes=True)
            ones_sb = cpool.tile([128, 1], f32, tag="ones_sb")
            nc.vector.memset(ones_sb[:], 1.0)
            sumexp_buf = cpool.tile([128, CHUNKS], bf16, tag="sumexp_buf")
            lse = cpool.tile([128, CHUNKS], f32, tag="lse")
            red = cpool.tile([128, 2], f32, tag="red")
            S_ps = apool.tile([MC, COLS], f32, tag="S_ps")

            pending_add = None

            def _emit_add(pa):
                ac0, an, aex = pa
                with nc.allow_low_precision("bf16 sumexp; rel tol 2e-2"):
                    nc.vector.tensor_reduce(
                        sumexp_buf[:, ac0:ac0 + an],
                        aex[:].rearrange("p (c k) -> p c k", k=C),
                        axis=AX.X, op=ALU.add)

            for c0, n in PIECES:
                pool = cbpool if n == 64 else cbspool
                cb = pool.tile([128, n * COLS], f8, tag=f"cb{n}")
                nc.sync.dma_start(cb[:], cbd[:, c0 * COLS:(c0 + n) * COLS])
                cb_r = cb[:].rearrange("p (c m) -> p c m", m=COLS)
                # labels as an int8 view of col 128 of each chunk
                labv = cb[:].bitcast(mybir.dt.int8).rearrange(
                    "p (c k) -> p c k", k=COLS)[:, :, LB:LB + 1]
                # onehot(label) for the piece's n*128 pixels, fp8 out
                # pad cols 21:32 are left as garbage: each stationary column
                # only feeds its own S_ps row, and rows 21:31 are discarded
                oh = wpool.tile([128, n * MC], f8, tag=f"oh{n}")
                oh_r = oh[:].rearrange("p (c k) -> p c k", k=MC)
                nc.vector.tensor_tensor(
                    oh_r[:, :, 0:C],
                    iota_sb[:, 0:C].unsqueeze(1).broadcast_to([128, n, C]),
                    labv.broadcast_to([128, n, C]),
                    op=ALU.is_equal)
                # CE pieces: exp(preds) on ACT; the piece's sumexp
                # reduce is deferred one iteration so the DVE never idles
                # waiting for ACT (eq(p) runs while exp(p) is in flight)
                ex = wpool.tile([128, n * C], bf16, tag=f"ex{n}")
                nc.scalar.activation(
                    ex[:].rearrange("p (c k) -> p c k", k=C),
                    cb_r[:, :, P0:P0 + C], AF.Exp)
                if pending_add is not None:
                    _emit_add(pending_add)
                pending_add = (c0, n, ex)
                # segment sums: S += oh^T @ [z | lab | preds], 2 chunks/matmul
                # (label-byte cols land in S cols 128:130, ignored on host)
                for i in range(n // 2):
                    dc = c0 // 2 + i
                    nc.tensor.matmul(
                        S_ps[:],
                        oh_r[:, 2 * i:2 * i + 2, :],
                        cb_r[:, 2 * i:2 * i + 2, 0:COLS],
                        start=(dc == 0), stop=(dc == DCHUNKS - 1),
                        perf_mode=PM.DoubleRow)

            if pending_add is not None:
                _emit_add(pending_add)
            # --- epilogue: slse = sum(ln(sumexp)); accum_out gives the
            # per-partition row sums for free (split A/B so the tail piece
            # dependency chain stays short)
            nc.scalar.activation(lse[:, 0:TAILC], sumexp_buf[:, 0:TAILC],
                                 AF.Ln, accum_out=red[:, 0:1])
            nc.scalar.activation(lse[:, TAILC:CHUNKS],
                                 sumexp_buf[:, TAILC:CHUNKS],
                                 AF.Ln, accum_out=red[:, 1:2])
            fin_ps = pspool.tile([1, 2], f32, tag="fin_ps")
            nc.tensor.matmul(fin_ps[:], ones_sb[:], red[:], start=True,
                             stop=True)
            out_sb = cpool.tile([C, COLS + 2], f32, tag="out_sb")
            nc.scalar.copy(out_sb[:, 0:COLS], S_ps[0:C, :])
            nc.scalar.copy(out_sb[0:1, COLS:COLS + 2], fin_ps[:])
            nc.sync.dma_start(outd, out_sb[:])

    nc.compile()
    _nc_cache = nc
    return nc


_F8 = ml_dtypes.float8_e4m3
_BF16 = ml_dtypes.bfloat16


def _make_in_maps(preds, labels, z, W_star):
    in_maps = []
    for i in range(NCORES):
        n, h0 = i // 2, (i % 2) * (H // 2)
        # pixel p = h*512 + w -> chunk = h*4 + w//128, lane = w%128
        zc = z[n, :, h0:h0 + H // 2, :].reshape(D, 256, 4, 128)
        zc = zc.transpose(3, 1, 2, 0).reshape(128, CHUNKS, D)
        pc = preds[n, :, h0:h0 + H // 2, :].reshape(C, 256, 4, 128)
        pc = pc.transpose(3, 1, 2, 0).reshape(128, CHUNKS, C)
        lc = labels[n, h0:h0 + H // 2, :].reshape(256, 4, 128)
        lc = lc.transpose(2, 0, 1).reshape(128, CHUNKS)
        combo = np.empty((128, CHUNKS, COLS), dtype=_F8)
        combo[:, :, 0:D] = zc.astype(_F8)
        combo[:, :, LB] = lc.astype(np.int8).view(np.uint8).view(_F8)
        combo[:, :, P0:P0 + C] = pc.astype(_F8)
        cpad = np.zeros((128, CHUNKS * COLS + 1024), dtype=_F8)
        cpad[:, :CHUNKS * COLS] = combo.reshape(128, CHUNKS * COLS)
        in_maps.append(dict(combo=cpad))
    return in_maps


def _combine(outs, W_star, counts, npix):
    """outs: 8x(21,153) blocks ([S_z|lab junk|SP|slse_a slse_b]) -> loss."""
    tot = np.sum([o.astype(np.float64) for o in outs], axis=0)
    S_z = tot[0:C, 0:D]
    SP = tot[0:C, P0:P0 + C]
    slse = tot[0, COLS] + tot[0, COLS + 1]
    ssx = SP.sum()
    sem = (slse - (1.0 - LS) * np.trace(SP) - (LS / C) * ssx) / npix
    cnt = counts.astype(np.float64)
    Zbar = np.where(cnt[:, None] > 0, S_z / np.maximum(cnt, 1.0)[:, None], 0.0)
    logits = Zbar @ W_star.astype(np.float64)
    m = logits.max(axis=1, keepdims=True)
    lse_r = m[:, 0] + np.log(np.exp(logits - m).sum(axis=1))
    lcr = np.mean(lse_r - (1.0 - LS) * np.diag(logits)
                  - (LS / C) * logits.sum(axis=1))
    return np.float32(LAMBDA_REG * lcr + sem)


def kernel(preds, labels, labels_depth, z, W_star):
    nc = _build()
    in_maps = _make_in_maps(preds, labels, z, W_star)
    res = bass_utils.run_bass_kernel_spmd(nc, in_maps,
                                          core_ids=list(range(NCORES)))
    counts = np.bincount(labels.reshape(-1), minlength=C).astype(np.float64)
    return _combine([r["out"] for r in res.results], W_star, counts,
                    float(labels.size))


if __name__ == "__main__":
    rng = np.random.default_rng(0)
    preds = rng.standard_normal((N, C, H, W), dtype=np.float32)
    labels = rng.integers(0, C, size=(N, H, W)).astype(np.int32)
    ld = rng.standard_normal((N, H, W), dtype=np.float32)
    z = rng.standard_normal((N, D, H, W), dtype=np.float32)
    Wst = rng.standard_normal((D, C), dtype=np.float32) * 0.3
    print("loss:", kernel(preds, labels, ld, z, Wst))


# revision 23
# speedup vs baseline: 1.1327x; 1.0080x over previous
"""Trainium2 Bass kernel for nn_LossCR (segment-reduce + dual CE loss).

Strategy (data-parallel over N x H/2 -> 8 shards of 131072 pixels):
  Host packs, per core, a pixel-major fp8(e4m3) "combo" tensor
  (128 lanes, 1024 chunks, 150 cols) = [z(128) | label_int8(1) |
  preds(21)].  Labels ride inside the combo stream (read on device via an
  int8 bitcast view of col 128) so every DMA queue gets identical work --
  a separate labels DMA skewed one queue by ~6us.  The DRAM row is padded
  by 1024B so the row stride stays 1024 mod 4096 (stride 0 mod 4096 made
  some queues' descriptors ~2x slower from HBM channel conflicts).
  Device, per 256-pixel double-chunk: one fp8 DoubleRow matmul
      S(21,150) += onehot(128,2,21)^T @ combo(128,2,150)
  PSUM-accumulated over all 512 double-chunks (the label col feeds an
  ignored S column).  Onehot built on DVE (is_equal vs int8 iota, fp8
  out, 32-col slabs whose junk pads feed only discarded S rows);
  exp(preds) on ACT; per-pixel sumexp reduce on DVE (bf16), emitted one
  piece behind the is_eq so the DVE never idles waiting for ACT;
  ln(sumexp) on ACT with accum_out giving the per-partition row sums for
  free.  A single manually-loaded ACT table holds BOTH exp and ln (no
  mid-kernel table switch).  Counts come from a host-side bincount.
  Piece schedule: 16/16-chunk lead-in (early DVE start), fifteen
  64-chunk body pieces on a deep 8-buffer DMA ring (absorbs HBM
  jitter), 16/16-chunk tail (short final DMA->exp->reduce->ln->out
  chain).
  Host: sum 8 partial (21,152) blocks ([S | slse_a slse_b] row 0),
  tiny (21,) softmax math in f64.
"""
import sys

sys.path.insert(0, "/opt/trn_rl_repo")
import numpy as np
import ml_dtypes
import concourse.bacc as bacc
import concourse.mybir as mybir
import concourse.tile as tile
from concourse import bass_utils
from concourse._compat import axon_active
from concourse.hw_specs import get_activation_tables

f32 = mybir.dt.float32
bf16 = mybir.dt.bfloat16
f8 = mybir.dt.float8e4
AF = mybir.ActivationFunctionType
ALU = mybir.AluOpType
AX = mybir.AxisListType
PM = mybir.MatmulPerfMode

N, C, H, W, D = 4, 21, 512, 512, 128
NCORES = 8
PIX = N * H * W // NCORES      # 131072 pixels per core
CHUNKS = PIX // 128            # 1024
DCHUNKS = CHUNKS // 2          # 512 double-chunks (DoubleRow: 256 px each)
LB = D                         # label byte (int8) at col 128
P0 = D + 1                     # preds start col
COLS = D + 1 + C               # 150
MC = 32                        # onehot cols per slab (DoubleRow needs 32-aligned)
LS = 0.1                       # label smoothing
LAMBDA_REG = 0.4
PIECES = ([(0, 16), (16, 16), (32, 32), (64, 32), (96, 32)]
          + [(128 + g * 64, 64) for g in range(13)]
          + [(960, 32), (992, 16), (1008, 16)])
TAILC = 992                    # chunks covered before the tail pieces

_nc_cache = None


def _build():
    global _nc_cache
    if _nc_cache is not None:
        return _nc_cache
    nc = bacc.Bacc("TRN2", target_bir_lowering=False, debug=not axon_active())
    # 1024B row pad keeps the DRAM row stride at 1024 mod 4096 so
    # partition rows spread across HBM channels (stride 0 mod 4096 made
    # some DMA queues' descriptors run ~2x slower from channel conflicts)
    cbd = nc.dram_tensor("combo", [128, CHUNKS * COLS + 1024], f8,
                         kind="ExternalInput").ap()
    outd = nc.dram_tensor("out", [C, COLS + 2], f32, kind="ExternalOutput").ap()

    with tile.TileContext(nc) as tc:
        with tc.tile_pool(name="const", bufs=1) as cpool, \
             tc.tile_pool(name="work", bufs=6) as wpool, \
             tc.tile_pool(name="cb", bufs=8) as cbpool, \
             tc.tile_pool(name="cbs", bufs=4) as cbspool, \
             tc.tile_pool(name="ps", bufs=1, space="PSUM") as pspool, \
             tc.tile_pool(name="acc", bufs=1, space="PSUM") as apool:
            # Preload the one ACT table that holds BOTH exp and ln so the
            # auto-inserter never needs a mid-kernel table switch.
            tabs = list(get_activation_tables(nc.m.arch).values())
            tid = next(i for i, s in enumerate(tabs)
                       if AF.Exp in s and AF.Ln in s)
            nc.scalar.add_instruction(mybir.InstLoadActFuncSet(
                name=nc.get_next_instruction_name(), ins=[], outs=[],
                act_func_set_id=tid))

            iota_sb = cpool.tile([128, MC], mybir.dt.int8, tag="iota_sb")
            nc.gpsimd.iota(iota_sb[:], [[1, MC]], channel_multiplier=0,
                           allow_small_or_imprecise_dtypes=True)
            ones_sb = cpool.tile([128, 1], f32, tag="ones_sb")
            nc.vector.memset(ones_sb[:], 1.0)
            sumexp_buf = cpool.tile([128, CHUNKS], bf16, tag="sumexp_buf")
            lse = cpool.tile([128, CHUNKS], f32, tag="lse")
            red = cpool.tile([128, 2], f32, tag="red")
            S_ps = apool.tile([MC, COLS], f32, tag="S_ps")

            pending_add = None

            def _emit_add(pa):
                ac0, an, aex = pa
                with nc.allow_low_precision("bf16 sumexp; rel tol 2e-2"):
                    nc.vector.tensor_reduce(
                        sumexp_buf[:, ac0:ac0 + an],
                        aex[:].rearrange("p (c k) -> p c k", k=C),
                        axis=AX.X, op=ALU.add)

            for c0, n in PIECES:
                pool = cbpool if n == 64 else cbspool
                cb = pool.tile([128, n * COLS], f8, tag=f"cb{n}")
                nc.sync.dma_start(cb[:], cbd[:, c0 * COLS:(c0 + n) * COLS])
                cb_r = cb[:].rearrange("p (c m) -> p c m", m=COLS)
                # labels as an int8 view of col 128 of each chunk
                labv = cb[:].bitcast(mybir.dt.int8).rearrange(
                    "p (c k) -> p c k", k=COLS)[:, :, LB:LB + 1]
                # onehot(label) for the piece's n*128 pixels, fp8 out
                # pad cols 21:32 are left as garbage: each stationary column
                # only feeds its own S_ps row, and rows 21:31 are discarded
                oh = wpool.tile([128, n * MC], f8, tag=f"oh{n}")
                oh_r = oh[:].rearrange("p (c k) -> p c k", k=MC)
                nc.vector.tensor_tensor(
                    oh_r[:, :, 0:C],
                    iota_sb[:, 0:C].unsqueeze(1).broadcast_to([128, n, C]),
                    labv.broadcast_to([128, n, C]),
                    op=ALU.is_equal)
                # CE pieces: exp(preds) on ACT; the piece's sumexp
                # reduce is deferred one iteration so the DVE never idles
                # waiting for ACT (eq(p) runs while exp(p) is in flight)
                ex = wpool.tile([128, n * C], bf16, tag=f"ex{n}")
                nc.scalar.activation(
                    ex[:].rearrange("p (c k) -> p c k", k=C),
                    cb_r[:, :, P0:P0 + C], AF.Exp)
                if pending_add is not None:
                    _emit_add(pending_add)
                pending_add = (c0, n, ex)
                # segment sums: S += oh^T @ [z | lab | preds], 2 chunks/matmul
                # (label-byte cols land in S cols 128:130, ignored on host)
                for i in range(n // 2):
                    dc = c0 // 2 + i
                    nc.tensor.matmul(
                        S_ps[:],
                        oh_r[:, 2 * i:2 * i + 2, :],
                        cb_r[:, 2 * i:2 * i + 2, 0:COLS],
                        start=(dc == 0), stop=(dc == DCHUNKS - 1),
                        perf_mode=PM.DoubleRow)

            if pending_add is not None:
                _emit_add(pending_add)
            # --- epilogue: slse = sum(ln(sumexp)); accum_out gives the
            # per-partition row sums for free (split A/B so the tail piece
            # dependency chain stays short)
            nc.scalar.activation(lse[:, 0:TAILC], sumexp_buf[:, 0:TAILC],
                                 AF.Ln, accum_out=red[:, 0:1])
            nc.scalar.activation(lse[:, TAILC:CHUNKS],
                                 sumexp_buf[:, TAILC:CHUNKS],
                                 AF.Ln, accum_out=red[:, 1:2])
            fin_ps = pspool.tile([1, 2], f32, tag="fin_ps")
            nc.tensor.matmul(fin_ps[:], ones_sb[:], red[:], start=True,
                             stop=True)
            out_sb = cpool.tile([C, COLS + 2], f32, tag="out_sb")
            nc.scalar.copy(out_sb[:, 0:COLS], S_ps[0:C, :])
            nc.scalar.copy(out_sb[0:1, COLS:COLS + 2], fin_ps[:])
            nc.sync.dma_start(outd, out_sb[:])

    nc.compile()
    _nc_cache = nc
    return nc


_F8 = ml_dtypes.float8_e4m3
_BF16 = ml_dtypes.bfloat16


def _make_in_maps(preds, labels, z, W_star):
    in_maps = []
    for i in range(NCORES):
        n, h0 = i // 2, (i % 2) * (H // 2)
        # pixel p = h*512 + w -> chunk = h*4 + w//128, lane = w%128
        zc = z[n, :, h0:h0 + H // 2, :].reshape(D, 256, 4, 128)
        zc = zc.transpose(3, 1, 2, 0).reshape(128, CHUNKS, D)
        pc = preds[n, :, h0:h0 + H // 2, :].reshape(C, 256, 4, 128)
        pc = pc.transpose(3, 1, 2, 0).reshape(128, CHUNKS, C)
        lc = labels[n, h0:h0 + H // 2, :].reshape(256, 4, 128)
        lc = lc.transpose(2, 0, 1).reshape(128, CHUNKS)
        combo = np.empty((128, CHUNKS, COLS), dtype=_F8)
        combo[:, :, 0:D] = zc.astype(_F8)
        combo[:, :, LB] = lc.astype(np.int8).view(np.uint8).view(_F8)
        combo[:, :, P0:P0 + C] = pc.astype(_F8)
        cpad = np.zeros((128, CHUNKS * COLS + 1024), dtype=_F8)
        cpad[:, :CHUNKS * COLS] = combo.reshape(128, CHUNKS * COLS)
        in_maps.append(dict(combo=cpad))
    return in_maps


def _combine(outs, W_star, counts, npix):
    """outs: 8x(21,153) blocks ([S_z|lab junk|SP|slse_a slse_b]) -> loss."""
    tot = np.sum([o.astype(np.float64) for o in outs], axis=0)
    S_z = tot[0:C, 0:D]
    SP = tot[0:C, P0:P0 + C]
    slse = tot[0, COLS] + tot[0, COLS + 1]
    ssx = SP.sum()
    sem = (slse - (1.0 - LS) * np.trace(SP) - (LS / C) * ssx) / npix
    cnt = counts.astype(np.float64)
    Zbar = np.where(cnt[:, None] > 0, S_z / np.maximum(cnt, 1.0)[:, None], 0.0)
    logits = Zbar @ W_star.astype(np.float64)
    m = logits.max(axis=1, keepdims=True)
    lse_r = m[:, 0] + np.log(np.exp(logits - m).sum(axis=1))
    lcr = np.mean(lse_r - (1.0 - LS) * np.diag(logits)
                  - (LS / C) * logits.sum(axis=1))
    return np.float32(LAMBDA_REG * lcr + sem)


def kernel(preds, labels, labels_depth, z, W_star):
    nc = _build()
    in_maps = _make_in_maps(preds, labels, z, W_star)
    res = bass_utils.run_bass_kernel_spmd(nc, in_maps,
                                          core_ids=list(range(NCORES)))
    counts = np.bincount(labels.reshape(-1), minlength=C).astype(np.float64)
    return _combine([r["out"] for r in res.results], W_star, counts,
                    float(labels.size))


if __name__ == "__main__":
    rng = np.random.default_rng(0)
    preds = rng.standard_normal((N, C, H, W), dtype=np.float32)
    labels = rng.integers(0, C, size=(N, H, W)).astype(np.int32)
    ld = rng.standard_normal((N, H, W), dtype=np.float32)
    z = rng.standard_normal((N, D, H, W), dtype=np.float32)
    Wst = rng.standard_normal((D, C), dtype=np.float32) * 0.3
    print("loss:", kernel(preds, labels, ld, z, Wst))
